# revision 24
# baseline (speedup 1.0000x reference)
"""Bivariate Gaussian kernel (Nadaraya-Watson) on 8 TRN2 NeuronCores.

Math: for query m, result[m] = t[m] / (s[m] + EPS) where
  w[n,m] = exp(-c[m] * d2[n,m]),  c[m] = 1/(2*bw[m]^2)
  s[m] = sum_n w[n,m],  t[m] = sum_n w[n,m]*outputs[n]

Device algorithm (per core, M_loc=1024 queries):
  exponent E[n,m] = P[m] + Q[m]*a2[n] + R[m]*in0[n] + S[m]*in1[n]
    (P=-c*b2, Q=-c, R=2c*x0, S=2c*x1) computed as rank-11 matmuls on the PE
    using error-compensated bf16 hi/lo splits (~1e-3 abs exact),
    with 3-4 n-tiles packed CONCURRENTLY into the 128x128 array via row
    tile_position (k=11 strips at rows 0/32/64/96 run simultaneously),
  W = exp(E) on the scalar engine (PSUM -> SBUF, bf16),
  [s; t_hi; t_lo] accumulated over n by a second matmul with stationary
    [ones, out_hi, out_lo] (bf16), PSUM accumulation across all 64 n-tiles;
    both m-half accumulators share one PSUM bank via col tile_position.
Queries (M) are sharded across the 8 cores; each core sees all N points.
"""

import functools
import sys

import numpy as np

sys.path.insert(0, "/opt/trn_rl_repo")

EPS = 1e-7
N = 8192
M = 8192
NCORES = 8
MLOC = M // NCORES  # 1024
P = 128
NT = N // P  # 64 n-tiles
MBW = 512  # m-block width (one PSUM bank)
MB = MLOC // MBW  # 2 m-blocks
NBLK = NT * MB  # 128 blocks of (128n x 512m)
K = 11  # compensated-split rank

# per-half n-tile grouping: alternates the 4-bank and 3-bank PSUM buffers
HALF_SIZES = [1, 2] + [4, 3] * 8 + [4, 1]
assert sum(HALF_SIZES) == NT and len(HALF_SIZES) % 2 == 0


def _half_groups():
    out = []
    pos = 0
    for sz in HALF_SIZES:
        out.append(list(range(pos, pos + sz)))
        pos += sz
    return out


@functools.lru_cache(maxsize=1)
def _build():
    import concourse.tile as tile
    from concourse import bacc, mybir

    f32 = mybir.dt.float32
    bf16 = mybir.dt.bfloat16
    EXP = mybir.ActivationFunctionType.Exp

    tgroups = _half_groups()
    NG = len(tgroups)  # col-slices in the packed stationary

    nc = bacc.Bacc("TRN2", target_bir_lowering=False, debug=False, num_devices=NCORES)
    # packed E stationary: band r (rows 32r..32r+10) of col-slice g holds the
    # A-rows of the r-th n-tile of group g. Rows outside the bands are unread.
    stat_d = nc.dram_tensor("stat", [P, NG * P], bf16, kind="ExternalInput")
    # E moving: every band holds the same 11 B-rows (PQRS hi/lo splits).
    mov_d = nc.dram_tensor("mov", [P, MLOC], bf16, kind="ExternalInput")
    rsb_d = nc.dram_tensor("rsb", [P, 4 * NT], bf16, kind="ExternalInput")
    res_d = nc.dram_tensor("res", [3 * MB, MBW], f32, kind="ExternalOutput")

    with tile.TileContext(nc) as tc:
        with (
            tc.tile_pool(name="const", bufs=1) as cpool,
            tc.tile_pool(name="w", bufs=4) as wpool,
            tc.tile_pool(name="epsum", bufs=1, space="PSUM") as epool,
            tc.tile_pool(name="acc", bufs=1, space="PSUM") as apool,
        ):
            # PE warm-up + exp-table preload on a never-written (garbage)
            # tile: no data deps, so both start right after the preamble and
            # run while the input DMAs stream. Results are never read.
            junk = cpool.tile([P, MBW], bf16, tag="junk")
            nc.gpsimd.memset(junk[0:1, 0:1], 0.0)
            ed = epool.tile([P, MBW * 4], f32, tag="e4")
            for _ in range(6):
                nc.tensor.matmul(
                    ed[:, 0:MBW], junk[:, 0:P], junk[:], start=True, stop=True
                )
            scr2 = cpool.tile([1, 8], f32, tag="scr2")
            nc.scalar.activation(scr2[:], junk[0:1, 0:8], EXP)

            # input loads: only the 11-row bands are ever read, so only those
            # rows are transferred — a few hundred KB total, split across the
            # gpsimd and scalar DMA queues.
            stat = cpool.tile([P, NG * P], bf16)
            mov = cpool.tile([P, MLOC], bf16)
            rsb = cpool.tile([P, 4 * NT], bf16)
            # full-image transfers: a 128-partition DMA uses all SBUF ports,
            # so shipping the zero filler rows is faster than 11-row bands.
            nc.scalar.dma_start(mov[:], mov_d[:])
            nc.scalar.dma_start(rsb[:], rsb_d[:])
            half = NG * P // 2
            nc.gpsimd.dma_start(stat[:, 0:half], stat_d[:, 0:half])
            nc.gpsimd.dma_start(stat[:, half:], stat_d[:, half:])

            # both m-half accumulators share one PSUM bank: rows [s;t_hi;t_lo]
            # at partitions 0-2 (m-lo) and 32-34 (m-hi, via col tile_position).
            acc = apool.tile([35, MBW], f32)

            started = [False] * MB
            pending = []

            def evict(h):
                st = cpool.tile([3, MBW], f32, tag=f"st{h}")
                nc.vector.tensor_copy(st[:], acc[32 * h : 32 * h + 3, :])
                nc.gpsimd.dma_start(res_d[3 * h : 3 * h + 3, :], st[:])

            def emit_reduce(w, h, tiles):
                for j, i in enumerate(tiles):
                    nc.tensor.matmul(
                        acc[32 * h : 32 * h + 3, :],
                        rsb[:, 4 * i : 4 * i + 3],
                        w[:, j * MBW : (j + 1) * MBW],
                        start=not started[h],
                        stop=i == NT - 1,
                        tile_position=(0, 32 * h),
                    )
                    started[h] = True
                if tiles[-1] == NT - 1:
                    evict(h)

            gi = 0
            for h in range(MB):
                for g, tiles in enumerate(tgroups):
                    if gi % 2 == 0:
                        e = epool.tile([P, MBW * 4], f32, tag="e4")
                    else:
                        e = epool.tile([P, MBW * 3], f32, tag="e3")
                    gi += 1
                    # packed concurrent E matmuls: strip r computes n-tile
                    # tiles[r] using array rows 32r..32r+10.
                    for r, i in enumerate(tiles):
                        nc.tensor.matmul(
                            e[:, r * MBW : (r + 1) * MBW],
                            stat[32 * r : 32 * r + K, g * P : (g + 1) * P],
                            mov[32 * r : 32 * r + K, h * MBW : (h + 1) * MBW],
                            start=True,
                            stop=True,
                            tile_position=(32 * r, 0),
                        )
                    w = wpool.tile([P, MBW * 4], bf16, tag="w")
                    fs = len(tiles) * MBW
                    nc.scalar.activation(w[:, :fs], e[:, :fs], EXP)
                    pending.append((w, h, tiles))
                    if len(pending) > 2:
                        emit_reduce(*pending.pop(0))
            for args in pending:
                emit_reduce(*args)

    nc.compile()
    return nc


def _bf16_split(v):
    import ml_dtypes

    hi = v.astype(ml_dtypes.bfloat16)
    lo = (v - hi.astype(np.float64)).astype(ml_dtypes.bfloat16)
    return hi, lo


def _prepare(x, inputs, outputs, bandwidth):
    """Host-side O(N+M) prep of the factored operands."""
    import ml_dtypes

    in0 = inputs[:, 0].astype(np.float64)
    in1 = inputs[:, 1].astype(np.float64)
    a2 = in0 * in0 + in1 * in1
    x0 = x[:, 0].astype(np.float64)
    x1 = x[:, 1].astype(np.float64)
    b2 = x0 * x0 + x1 * x1
    c = 1.0 / (2.0 * bandwidth.astype(np.float64) ** 2)
    Pm = -c * b2
    Qm = -c
    Rm = 2.0 * c * x0
    Sm = 2.0 * c * x1

    ones = np.ones(N, np.float64)
    a2h, a2l = _bf16_split(a2)
    i0h, i0l = _bf16_split(in0)
    i1h, i1l = _bf16_split(in1)
    oneh, _ = _bf16_split(ones)
    Ph, Pl = _bf16_split(Pm)
    Qh, Ql = _bf16_split(Qm)
    Rh, Rl = _bf16_split(Rm)
    Sh, Sl = _bf16_split(Sm)

    # row pairing: E = P(hi+lo) + a2hi*Q(hi+lo) + a2lo*Qhi + (same for in0,in1)
    stat_rows = np.stack(
        [oneh, oneh, a2h, a2h, a2l, i0h, i0h, i0l, i1h, i1h, i1l]
    )  # (K, N)
    mov_rows = np.stack([Ph, Pl, Qh, Ql, Qh, Rh, Rl, Rh, Sh, Sl, Sh])  # (K, M)

    tgroups = _half_groups()
    NG = len(tgroups)
    stat = np.zeros((P, NG * P), ml_dtypes.bfloat16)
    for g, tiles in enumerate(tgroups):
        for r, i in enumerate(tiles):
            stat[32 * r : 32 * r + K, g * P : (g + 1) * P] = stat_rows[
                :, i * P : (i + 1) * P
            ]
    mov = np.zeros((P, M), ml_dtypes.bfloat16)
    for r in range(4):
        mov[32 * r : 32 * r + K, :] = mov_rows

    oh, ol = _bf16_split(outputs.astype(np.float64))
    rsb = np.zeros((N, 4), ml_dtypes.bfloat16)
    rsb[:, 0] = 1.0
    rsb[:, 1] = oh
    rsb[:, 2] = ol
    # per n-tile lhsT layout: rsb_sb[p, 4i+c] = rsb[i*128+p, c]
    rsb_sb = np.ascontiguousarray(
        rsb.reshape(NT, P, 4).transpose(1, 0, 2).reshape(P, 4 * NT)
    )
    return stat, mov, rsb_sb


def kernel(x, inputs, outputs, bandwidth):
    from concourse.bass_utils import run_bass_kernel_spmd

    x = np.asarray(x, np.float32)
    inputs = np.asarray(inputs, np.float32)
    outputs = np.asarray(outputs, np.float32)
    bandwidth = np.asarray(bandwidth, np.float32)

    stat, mov, rsb_sb = _prepare(x, inputs, outputs, bandwidth)

    nc = _build()
    in_maps = [
        {
            "stat": stat,
            "mov": np.ascontiguousarray(mov[:, c * MLOC : (c + 1) * MLOC]),
            "rsb": rsb_sb,
        }
        for c in range(NCORES)
    ]
    res = run_bass_kernel_spmd(nc, in_maps, list(range(NCORES)))
    parts = []
    for c in range(NCORES):
        st = res.results[c]["res"]  # (6,512): [s,t_hi,t_lo] x {m-lo, m-hi}
        s = np.concatenate([st[0], st[3]])
        t = np.concatenate([st[1] + st[2], st[4] + st[5]])
        parts.append(t / (s + EPS))
    return np.concatenate(parts).astype(np.float32)


if __name__ == "__main__":
    rng = np.random.default_rng(0)
    x = rng.standard_normal((M, 2), np.float32)
    inputs = rng.standard_normal((N, 2), np.float32)
    outputs = rng.standard_normal(N, np.float32)
    bandwidth = (0.5 + rng.random(M)).astype(np.float32)
    got = kernel(x, inputs, outputs, bandwidth)
    print(got[:8])
